# revision 1
# baseline (speedup 1.0000x reference)
"""Trainium2 Bass kernel for nn_CoreAttention (S=2048, B=1, H=16, D=128).

Sharding: 16 heads across 8 NeuronCores (2 heads/core, tensor parallel).

Per head (big tensors stay feature-major so nothing large is transposed
on device; the host supplies Q^T/K^T/V^T per head):
    qT     = (Wqk^T Q^T) / NF            (bf16 PE, fp32 PSUM)
    kT     = Wqk^T K^T                   (bf16 PE)
    scoresT[k,q] = kT-block^T @ qT       (bf16 PE; causal: only q >= k)
    scoresT += causal mask on diag block (PE accumulate of -1e4 tile)
    expT   = exp(scoresT)                (ACT, PSUM->SBUF bf16)
    sums[q]= ones-matmuls over expT      (PE, N=1 column sums)
    v      = V^T-chunks^T @ Wv           (bf16 PE -> natural [s,e] layout)
    ctxT   = sum_j v_j^T @ expT_j        (bf16 PE, fp32 accum)
    ctx    = transpose(ctxT) * (1/sums)  (fp32 PE transpose + DVE scale)

exp() runs without max-subtraction: scores are ~N(0,1) (the reference
normalizes by sqrt(128)), so exp never overflows and matches the
reference's masked softmax to rounding error.
"""

import sys
from contextlib import ExitStack

import numpy as np

for _p in ("/opt/trn_rl_repo",):
    if _p not in sys.path:
        sys.path.insert(0, _p)

import ml_dtypes
import concourse.bass as bass
import concourse.tile as tile
from concourse import bacc, mybir
from concourse.bass_utils import run_bass_kernel_spmd

S, B, H, D = 2048, 1, 16, 128
HPC = 2  # heads per core
NCORES = 8
NB = S // 128  # 16 seq blocks of 128
NF = float(np.sqrt(2048.0 / 16.0))  # NORM_FACTOR
NEG = -10000.0
PAD = 384  # zero-pad columns in front of each expt_j buffer

F32 = mybir.dt.float32
BF16 = mybir.dt.bfloat16
AF = mybir.ActivationFunctionType


def build_program() -> bass.Bass:
    nc = bacc.Bacc(
        "TRN2", target_bir_lowering=False, debug=False, num_devices=NCORES
    )

    qt_d = nc.dram_tensor("qt", [HPC, D, S], F32, kind="ExternalInput")
    kt_d = nc.dram_tensor("kt", [HPC, D, S], F32, kind="ExternalInput")
    vt_d = nc.dram_tensor("vt", [HPC, D, S], F32, kind="ExternalInput")
    wqk_d = nc.dram_tensor("wqk", [HPC, D, D], F32, kind="ExternalInput")
    wv_d = nc.dram_tensor("wv", [HPC, D, D], F32, kind="ExternalInput")
    identf_d = nc.dram_tensor("identf", [D, D], F32, kind="ExternalInput")
    identb_d = nc.dram_tensor("identb", [D, D], BF16, kind="ExternalInput")
    maskb_d = nc.dram_tensor("maskb", [D, D], BF16, kind="ExternalInput")
    onesb_d = nc.dram_tensor("onesb", [D, 1], BF16, kind="ExternalInput")
    onesf_d = nc.dram_tensor("onesf", [1, 1], F32, kind="ExternalInput")
    out_d = nc.dram_tensor("out", [HPC, S, D], F32, kind="ExternalOutput")

    with tile.TileContext(nc) as tc, ExitStack() as ctx:
        cpool = ctx.enter_context(tc.tile_pool(name="const", bufs=1))
        sb = ctx.enter_context(tc.tile_pool(name="sb", bufs=1))
        ps = ctx.enter_context(tc.tile_pool(name="ps", bufs=1, space="PSUM"))

        identf = cpool.tile([D, D], F32)
        nc.sync.dma_start(identf[:], identf_d[:])
        identb = cpool.tile([D, D], BF16)
        nc.sync.dma_start(identb[:], identb_d[:])
        maskb = cpool.tile([D, D], BF16)
        nc.sync.dma_start(maskb[:], maskb_d[:])
        onesb = cpool.tile([D, 1], BF16)
        nc.sync.dma_start(onesb[:], onesb_d[:])
        onesf = cpool.tile([1, 1], F32)
        nc.sync.dma_start(onesf[:], onesf_d[:])

        # Warm the PE's view of identf's DMA queue so later fp32 transposes
        # (self-loading, max 1 sync wait) never need a second wait.
        warm_ps = ps.tile([D, D], F32, tag="otr", name="warm_ps")
        nc.tensor.transpose(warm_ps[:], identf[:], identf[:])

        for h in range(HPC):
            # ---- load raw inputs (weights first: tiny, unblock projs) -----
            wqk = sb.tile([D, D], F32, tag="wqk", bufs=2)
            nc.sync.dma_start(wqk[:], wqk_d[h])
            wv = sb.tile([D, D], F32, tag="wv", bufs=2)
            nc.sync.dma_start(wv[:], wv_d[h])
            wqkb = sb.tile([D, D], BF16, tag="wqkb", bufs=2)
            nc.vector.tensor_copy(wqkb[:], wqk[:])
            wvb = sb.tile([D, D], BF16, tag="wvb", bufs=2)
            nc.vector.tensor_copy(wvb[:], wv[:])

            # q/k/v loads + bf16 casts, pipelined at 1024-col granularity
            qtr = sb.tile([D, S], F32, tag="qtr", bufs=2)
            ktr = sb.tile([D, S], F32, tag="ktr", bufs=2)
            vtr = sb.tile([D, S], F32, tag="vtr", bufs=2)
            qtb = sb.tile([D, S], BF16, tag="qtb", bufs=1)
            ktb = sb.tile([D, S], BF16, tag="ktb", bufs=1)
            vtb = sb.tile([D, S], BF16, tag="vtb", bufs=1)
            for raw, dr, cast in ((qtr, qt_d, qtb), (ktr, kt_d, ktb), (vtr, vt_d, vtb)):
                for c in range(2):
                    sl = slice(c * 1024, (c + 1) * 1024)
                    nc.sync.dma_start(raw[:, sl], dr[h][:, sl])
                    nc.vector.tensor_copy(cast[:, sl], raw[:, sl])

            # ---- projections: qT = Wqk^T Q^T / NF,  kT = Wqk^T K^T --------
            qmt = sb.tile([D, S], BF16, tag="qmt", bufs=2)
            kmt = sb.tile([D, S], BF16, tag="kmt", bufs=2)
            for src, dst, scale in ((qtb, qmt, 1.0 / NF), (ktb, kmt, 1.0)):
                for c in range(2):
                    p = ps.tile(
                        [D, S // 2], F32, tag="big", bufs=2,
                        name=f"proj_ps_{h}_{dst.tensor.name}_{c}",
                    )
                    for c2 in range(2):
                        nc.tensor.matmul(
                            p[:, c2 * 512 : (c2 + 1) * 512],
                            wqkb[:],
                            src[:, c * 1024 + c2 * 512 : c * 1024 + (c2 + 1) * 512],
                        )
                    nc.scalar.activation(
                        dst[:, c * 1024 : (c + 1) * 1024], p[:], AF.Copy, scale=scale
                    )

            # ---- v chunks in natural [s,e] layout: v = V_raw @ Wv ---------
            vsb = sb.tile([D, NB * D], BF16, tag="vsb", bufs=2)
            for c in range(2):
                vp = ps.tile([D, S // 2], F32, tag="big", bufs=2, name=f"vp_ps_{h}_{c}")
                for j in range(8):
                    nc.tensor.matmul(
                        vp[:, j * 128 : (j + 1) * 128],
                        vtb[:, (c * 8 + j) * 128 : (c * 8 + j + 1) * 128],
                        wvb[:],
                    )
                nc.vector.tensor_copy(vsb[:, c * 1024 : (c + 1) * 1024], vp[:])

            # ---- pass 1: scoresT -> exp(bf16), left-padded with zeros -----
            # expt_j buffer holds PAD zero columns then the w real columns,
            # so later N=512 reads spanning "before the diagonal" see zeros.
            expts = []
            for j in range(NB):
                w = S - j * 128  # sq columns j*128 .. S
                expt = sb.tile(
                    [D, PAD + w], BF16, tag=f"expt{j}", bufs=2, name=f"expt_h{h}_{j}"
                )
                nc.gpsimd.memset(expt[:, 0:PAD], 0.0)
                nhalf = (w + 1023) // 1024
                for c in range(nhalf):
                    lo = c * 1024
                    cw = min(1024, w - lo)
                    sc_ps = ps.tile(
                        [D, cw], F32, tag="big", bufs=2, name=f"sc_ps_h{h}_{j}_{c}"
                    )
                    for c2 in range(0, cw, 512):
                        ce = min(c2 + 512, cw)
                        first = c == 0 and c2 == 0
                        nc.tensor.matmul(
                            sc_ps[:, c2:ce],
                            kmt[:, j * 128 : (j + 1) * 128],
                            qmt[:, j * 128 + lo + c2 : j * 128 + lo + ce],
                            start=True,
                            stop=not first,
                        )
                        if first:
                            # causal mask on diagonal block via PE accumulate
                            nc.tensor.matmul(
                                sc_ps[:, 0:128],
                                identb[:],
                                maskb[:],
                                start=False,
                                stop=True,
                            )
                    nc.scalar.activation(
                        expt[:, PAD + lo : PAD + lo + cw], sc_ps[:], AF.Exp
                    )
                expts.append(expt)

            # ---- softmax sums: ones-stationary N=512 row-sums -------------
            recip_ps = ps.tile([D, NB], F32, tag="recipps", name=f"recip_ps_{h}")
            for c in range(4):
                srow = ps.tile([1, 512], F32, tag="sumsrow", name=f"srow_{h}_{c}")
                njc = 4 * c + 4  # j = 0 .. 4c+3 contribute to this chunk
                for j in range(njc):
                    nc.tensor.matmul(
                        srow[:],
                        onesb[:],
                        expts[j][:, PAD + 512 * c - 128 * j : PAD + 512 * c - 128 * j + 512],
                        start=(j == 0),
                        stop=(j == njc - 1),
                    )
                srow_sb = sb.tile([1, 512], F32, tag="srow_sb", bufs=2)
                nc.vector.tensor_copy(srow_sb[:], srow[:])
                for s4 in range(4):
                    i = c * 4 + s4
                    # [1,128] row -> [128,1] column via K=1 matmul
                    nc.tensor.matmul(
                        recip_ps[:, i : i + 1],
                        srow_sb[0:1, s4 * 128 : (s4 + 1) * 128],
                        onesf[:],
                    )
            recip = sb.tile([D, NB], F32, tag="recip", bufs=2)
            nc.vector.reciprocal(recip[:], recip_ps[:])

            # ---- pass 2: PV accumulation, transpose, normalize, store -----
            for i4 in range(NB // 4):
                outt_ps = ps.tile([D, 512], F32, tag="outt", name=f"outt_{h}_{i4}")
                njc = 4 * i4 + 4
                for j in range(njc):
                    nc.tensor.matmul(
                        outt_ps[:],
                        vsb[:, j * 128 : (j + 1) * 128],
                        expts[j][:, PAD + 512 * i4 - 128 * j : PAD + 512 * i4 - 128 * j + 512],
                        start=(j == 0),
                        stop=(j == njc - 1),
                    )
                outt_sb = sb.tile([D, 512], F32, tag="outt_sb", bufs=2)
                nc.vector.tensor_copy(outt_sb[:], outt_ps[:])
                otr_ps = ps.tile([D, 512], F32, tag="otr", name=f"otr_{h}_{i4}")
                osb = sb.tile([D, 512], F32, tag="osb", bufs=2)
                for s4 in range(4):
                    i = i4 * 4 + s4
                    sl = slice(s4 * 128, (s4 + 1) * 128)
                    nc.tensor.transpose(otr_ps[:, sl], outt_sb[:, sl], identf[:])
                    nc.vector.tensor_scalar_mul(
                        osb[:, sl], otr_ps[:, sl], recip[:, i : i + 1]
                    )
                nc.sync.dma_start(
                    out_d[h, i4 * 512 : (i4 + 1) * 512, :].rearrange(
                        "(b s) e -> s b e", b=4
                    ),
                    osb[:].rearrange("p (b e) -> p b e", b=4),
                )

    nc.compile()
    return nc


_NC_CACHE = None


def _get_program():
    global _NC_CACHE
    if _NC_CACHE is None:
        _NC_CACHE = build_program()
    return _NC_CACHE


def make_in_maps(query_layer, key_layer, value_layer, svd_qk, svd_v):
    qt = np.ascontiguousarray(query_layer[:, 0].transpose(1, 2, 0))
    kt = np.ascontiguousarray(key_layer[:, 0].transpose(1, 2, 0))
    vt = np.ascontiguousarray(value_layer[:, 0].transpose(1, 2, 0))
    svd_qk = np.ascontiguousarray(svd_qk, dtype=np.float32)
    svd_v = np.ascontiguousarray(svd_v, dtype=np.float32)

    identf = np.eye(D, dtype=np.float32)
    identb = np.eye(D, dtype=ml_dtypes.bfloat16)
    r = np.arange(D)
    maskb = np.where(r[:, None] > r[None, :], NEG, 0.0).astype(ml_dtypes.bfloat16)
    onesb = np.ones((D, 1), dtype=ml_dtypes.bfloat16)

    in_maps = []
    for c in range(NCORES):
        hs = slice(c * HPC, (c + 1) * HPC)
        in_maps.append(
            {
                "qt": qt[hs],
                "kt": kt[hs],
                "vt": vt[hs],
                "wqk": svd_qk[hs],
                "wv": svd_v[hs],
                "identf": identf,
                "identb": identb,
                "maskb": maskb,
                "onesb": onesb,
                "onesf": np.ones((1, 1), dtype=np.float32),
            }
        )
    return in_maps


def assemble_output(results):
    out = np.empty((S, B, H * D), dtype=np.float32)
    for c in range(NCORES):
        o = results[c]["out"]  # [HPC, S, D]
        for hl in range(HPC):
            h = c * HPC + hl
            out[:, 0, h * D : (h + 1) * D] = o[hl]
    return out


def kernel(query_layer, key_layer, value_layer, attention_mask, svd_qk, svd_v):
    nc = _get_program()
    in_maps = make_in_maps(query_layer, key_layer, value_layer, svd_qk, svd_v)
    res = run_bass_kernel_spmd(nc, in_maps, list(range(NCORES))).results
    return assemble_output(res)



# revision 2
# speedup vs baseline: 1.5100x; 1.5100x over previous
"""Trainium2 Bass kernel for nn_CoreAttention (S=2048, B=1, H=16, D=128).

Sharding: 16 heads across 8 NeuronCores (2 heads/core, tensor parallel).

Per head, fully fused causal attention:
    M      = Wqk Wqk^T                  (PE, one matmul; M is symmetric)
    kmt    = M K^T                      (PE, 4 matmuls; q side stays RAW)
    v      = V_block @ Wv               (PE, 16 matmuls -> [s,e] chunks)
    scoresT[k,q] = kmt_j^T @ Q^T        (PE, causal only, streams into two
                                         big PSUM spans: P=4 banks, Q=3)
    expT   = exp(scoresT / NF)          (ACT, ~10 big instrs/head, -> SBUF)
    mask   = affine_select on diagonal  (GPSIMD, zero upper triangle)
    ctx[q,(e|sum)] = sum_j expT_j^T @ [v_j | 1]   (PE, expT-stationary,
                                         129-wide rhs; col 128 = softmax sum)
    out    = ctx * (1/sum)              (DVE reciprocal + per-partition mul)

No transposes, no separate softmax-sum pass, no device-side casts (host
supplies bf16 pre-transposed tensors). exp runs without max-subtraction:
scores/NF ~ N(0,1), so exp stays in [e-6, e+6].
"""

import sys
from contextlib import ExitStack

import numpy as np

for _p in ("/opt/trn_rl_repo",):
    if _p not in sys.path:
        sys.path.insert(0, _p)

import ml_dtypes
import concourse.bass as bass
import concourse.tile as tile
from concourse import bacc, mybir
from concourse.bass_utils import run_bass_kernel_spmd

S, B, H, D = 2048, 1, 16, 128
HPC = 2  # heads per core
NCORES = 8
NB = S // 128  # 16 k-blocks of 128
NF = float(np.sqrt(2048.0 / 16.0))  # NORM_FACTOR
TOT = NB * (S + 128) // 2 // 128 * 128  # total causal score columns = 17408

F32 = mybir.dt.float32
BF16 = mybir.dt.bfloat16
AF = mybir.ActivationFunctionType

# block start offsets in the concatenated causal score stream
OFF = [0]
for j in range(NB):
    OFF.append(OFF[-1] + (S - 128 * j))
assert OFF[-1] == TOT

# exp spans: greedy alternation between PSUM regions P (2048 cols) and
# Q (1536 cols); each span is one big ACT exp instruction.
SPANS = []  # (region_idx 0/1, start_pos, length)
_pos = 0
_r = 0
_SZ = (2048, 1536)
while _pos < TOT:
    ln = min(_SZ[_r], TOT - _pos)
    SPANS.append((_r, _pos, ln))
    _pos += ln
    _r ^= 1


def build_program() -> bass.Bass:
    nc = bacc.Bacc(
        "TRN2", target_bir_lowering=False, debug=False, num_devices=NCORES
    )

    qt_d = nc.dram_tensor("qt", [HPC, D, S], BF16, kind="ExternalInput")
    kt_d = nc.dram_tensor("kt", [HPC, D, S], BF16, kind="ExternalInput")
    vt_d = nc.dram_tensor("vt", [HPC, D, S], BF16, kind="ExternalInput")
    wqkt_d = nc.dram_tensor("wqkt", [HPC, D, D], BF16, kind="ExternalInput")
    wv_d = nc.dram_tensor("wv", [HPC, D, D], BF16, kind="ExternalInput")
    out_d = nc.dram_tensor("out", [HPC, S, D], F32, kind="ExternalOutput")

    with tile.TileContext(nc) as tc, ExitStack() as ctx:
        sb = ctx.enter_context(tc.tile_pool(name="sb", bufs=1))
        ps = ctx.enter_context(tc.tile_pool(name="ps", bufs=1, space="PSUM"))

        # warm the exp activation-table load under the initial DMAs
        warm = sb.tile([D, 1], F32, tag="warm")
        nc.gpsimd.memset(warm[:], 0.0)
        warm2 = sb.tile([D, 1], BF16, tag="warm2")
        nc.scalar.activation(warm2[:], warm[:], AF.Exp)

        for h in range(HPC):
            # ---- input loads (weights first; all bf16 from host) ----------
            wqkt = sb.tile([D, D], BF16, tag="wqkt", bufs=2)
            nc.sync.dma_start(wqkt[:], wqkt_d[h])
            wvb = sb.tile([D, D], BF16, tag="wvb", bufs=2)
            nc.sync.dma_start(wvb[:], wv_d[h])
            ktb = sb.tile([D, S], BF16, tag="ktb", bufs=2)
            nc.sync.dma_start(ktb[:], kt_d[h])
            vtb = sb.tile([D, S], BF16, tag="vtb", bufs=2)
            nc.sync.dma_start(vtb[:], vt_d[h])
            qtb = sb.tile([D, S], BF16, tag="qtb", bufs=2)
            nc.sync.dma_start(qtb[:], qt_d[h])

            # PSUM: P = 4 banks, Q = 3 banks, ctx = 1 bank (3 slots x 129)
            P = ps.tile([D, 2048], F32, tag="P", name=f"P_{h}")
            Qr = ps.tile([D, 1536], F32, tag="Q", name=f"Q_{h}")
            ctxb = ps.tile([D, 3 * 129], F32, tag="ctx", name=f"ctx_{h}")
            regions = (P, Qr)

            # ---- prologue: M = W W^T, kmt = M K^T, v chunks ---------------
            nc.tensor.matmul(Qr[:, 0:128], wqkt[:], wqkt[:])
            Mb = sb.tile([D, D], BF16, tag="Mb", bufs=2)
            nc.vector.tensor_copy(Mb[:], Qr[:, 0:128])
            for c in range(4):
                sl = slice(512 * c, 512 * (c + 1))
                nc.tensor.matmul(P[:, sl], Mb[:], ktb[:, sl])
            kmt = sb.tile([D, S], BF16, tag="kmt", bufs=2)
            nc.vector.tensor_copy(kmt[:], P[:])
            for j in range(NB):
                nc.tensor.matmul(
                    P[:, 128 * j : 128 * (j + 1)],
                    vtb[:, 128 * j : 128 * (j + 1)],
                    wvb[:],
                )
            # vsb chunks are 129 wide: cols 0..127 = v_j, col 128 = ones
            vsb = sb.tile([D, NB * 129], BF16, tag="vsb", bufs=2)
            vsb3 = vsb.rearrange("p (j e) -> p j e", j=NB)
            nc.vector.tensor_copy(
                vsb3[:, :, 0:128], P.rearrange("p (j e) -> p j e", j=NB)
            )
            nc.gpsimd.memset(vsb3[:, :, 128:129], 1.0)

            expt = sb.tile([D, TOT], BF16, tag="expt", bufs=2)

            # ---- fused causal loop ----------------------------------------
            # walk the concatenated score stream; emit score matmuls, fire
            # exp per span, mask diagonals, then PV regions as they unlock.
            span_i = 0  # next span to exp
            pv_i = 0  # next PV region to emit
            osb = None

            def fire_span(si):
                r, st, ln = SPANS[si]
                reg = regions[r]
                nc.scalar.activation(
                    expt[:, st : st + ln], reg[:, 0:ln], AF.Exp, scale=1.0 / NF
                )
                # mask diag blocks living inside this span
                for j in range(NB):
                    if st <= OFF[j] < st + ln:
                        nc.gpsimd.affine_select(
                            expt[:, OFF[j] : OFF[j] + 128],
                            expt[:, OFF[j] : OFF[j] + 128],
                            pattern=[[1, 128]],
                            compare_op=mybir.AluOpType.is_ge,
                            fill=0.0,
                            base=0,
                            channel_multiplier=-1,
                        )

            def emit_pv(i):
                nonlocal osb
                slot = ctxb[:, (i % 3) * 129 : (i % 3) * 129 + 129]
                for j2 in range(i + 1):
                    woff = OFF[j2] + (i - j2) * 128
                    nc.tensor.matmul(
                        slot,
                        expt[:, woff : woff + 128],
                        vsb[:, 129 * j2 : 129 * j2 + 129],
                        start=(j2 == 0),
                        stop=(j2 == i),
                    )
                if i % 4 == 0:
                    osb = sb.tile([D, 512], F32, tag="osb", bufs=2,
                                  name=f"osb_{h}_{i // 4}")
                rec = sb.tile([D, 1], F32, tag="rec", bufs=3,
                              name=f"rec_{h}_{i}")
                nc.vector.reciprocal(rec[:], slot[:, 128:129])
                nc.vector.tensor_scalar_mul(
                    osb[:, (i % 4) * 128 : (i % 4) * 128 + 128],
                    slot[:, 0:128],
                    rec[:],
                )
                if i % 4 == 3:
                    g = i // 4
                    nc.sync.dma_start(
                        out_d[h, 512 * g : 512 * (g + 1), :].rearrange(
                            "(b s) e -> s b e", b=4
                        ),
                        osb.rearrange("p (b e) -> p b e", b=4),
                    )

            for j in range(NB):
                pos = OFF[j]
                end = OFF[j + 1]
                while pos < end:
                    r, st, ln = SPANS[span_i]
                    # chunk: up to block end, span end, next 512 boundary
                    nxt = min(end, st + ln, st + ((pos - st) // 512 + 1) * 512)
                    nc.tensor.matmul(
                        regions[r][:, pos - st : nxt - st],
                        kmt[:, 128 * j : 128 * (j + 1)],
                        qtb[:, 128 * j + (pos - OFF[j]) : 128 * j + (nxt - OFF[j])],
                    )
                    pos = nxt
                    if nxt == st + ln:  # span filled -> exp + masks
                        fire_span(span_i)
                        span_i += 1
                        # release PV regions whose weights are all exp'd
                        while pv_i < NB and OFF[pv_i] + 128 <= st + ln:
                            emit_pv(pv_i)
                            pv_i += 1
            while pv_i < NB:
                emit_pv(pv_i)
                pv_i += 1

    nc.compile()
    return nc


_NC_CACHE = None


def _get_program():
    global _NC_CACHE
    if _NC_CACHE is None:
        _NC_CACHE = build_program()
    return _NC_CACHE


def make_in_maps(query_layer, key_layer, value_layer, svd_qk, svd_v):
    bf = ml_dtypes.bfloat16
    qt = np.ascontiguousarray(
        np.asarray(query_layer)[:, 0].transpose(1, 2, 0).astype(bf)
    )
    kt = np.ascontiguousarray(
        np.asarray(key_layer)[:, 0].transpose(1, 2, 0).astype(bf)
    )
    vt = np.ascontiguousarray(
        np.asarray(value_layer)[:, 0].transpose(1, 2, 0).astype(bf)
    )
    wqkt = np.ascontiguousarray(
        np.asarray(svd_qk).transpose(0, 2, 1).astype(bf)
    )
    wv = np.ascontiguousarray(np.asarray(svd_v).astype(bf))

    in_maps = []
    for c in range(NCORES):
        hs = slice(c * HPC, (c + 1) * HPC)
        in_maps.append(
            {
                "qt": qt[hs],
                "kt": kt[hs],
                "vt": vt[hs],
                "wqkt": wqkt[hs],
                "wv": wv[hs],
            }
        )
    return in_maps


def assemble_output(results):
    out = np.empty((S, B, H * D), dtype=np.float32)
    for c in range(NCORES):
        o = results[c]["out"]  # [HPC, S, D]
        for hl in range(HPC):
            h = c * HPC + hl
            out[:, 0, h * D : (h + 1) * D] = o[hl]
    return out


def kernel(query_layer, key_layer, value_layer, attention_mask, svd_qk, svd_v):
    nc = _get_program()
    in_maps = make_in_maps(query_layer, key_layer, value_layer, svd_qk, svd_v)
    res = run_bass_kernel_spmd(nc, in_maps, list(range(NCORES))).results
    return assemble_output(res)


# revision 3
# speedup vs baseline: 1.6643x; 1.1022x over previous
"""Trainium2 Bass kernel for nn_CoreAttention (S=2048, B=1, H=16, D=128).

Sharding: 16 heads across 8 NeuronCores (2 heads/core, tensor parallel).

Per head, fully fused causal attention:
    M      = Wqk Wqk^T                  (PE, one matmul; M is symmetric)
    kmt    = M K^T                      (PE, 4 matmuls ping-ponged through a
                                         1-bank PSUM scratch; q side stays RAW)
    v      = V_block @ Wv               (PE, 16 matmuls -> [s,e] chunks)
    scoresT[k,q] = kmt_j^T @ Q^T        (PE, causal only, streams into two
                                         big PSUM spans: P=4 banks, Q=2)
    expT   = exp(scoresT / NF)          (ACT, 12 big instrs/head, -> SBUF)
    mask   = affine_select on diagonal  (GPSIMD, zero strict lower triangle)
    ctx[q,(e|sum)] = sum_j expT_j^T @ [v_j | 1]   (PE, expT-stationary,
                                         129-wide rhs; col 128 = softmax sum)
    out    = ctx * (1/sum)              (DVE reciprocal + per-partition mul)

The two heads are software-pipelined: head1's projections are interleaved
into head0's span loop so ACT/PE never drain at the head seam. Input DMAs
are issued up-front on two queues (sync + gpsimd). No transposes, no
separate softmax-sum pass, no device-side casts (host supplies bf16
pre-transposed tensors). exp runs without max-subtraction: scores/NF ~
N(0,1), so exp stays in [e-6, e+6].
"""

import sys
from contextlib import ExitStack

import numpy as np

for _p in ("/opt/trn_rl_repo",):
    if _p not in sys.path:
        sys.path.insert(0, _p)

import ml_dtypes
import concourse.bass as bass
import concourse.tile as tile
from concourse import bacc, mybir
from concourse.bass_utils import run_bass_kernel_spmd

S, B, H, D = 2048, 1, 16, 128
HPC = 2  # heads per core
NCORES = 8
NB = S // 128  # 16 k-blocks of 128
NF = float(np.sqrt(2048.0 / 16.0))  # NORM_FACTOR
TOT = 17408  # total causal score columns

F32 = mybir.dt.float32
BF16 = mybir.dt.bfloat16
AF = mybir.ActivationFunctionType

# block start offsets in the concatenated causal score stream
OFF = [0]
for j in range(NB):
    OFF.append(OFF[-1] + (S - 128 * j))
assert OFF[-1] == TOT


def make_spans(split_last: bool):
    """(region_idx, region_off, start, len) alternating P(2048) / Q(1024)."""
    sizes = [2048, 1024] * 4 + [2048, 1024, 1024, 1024]
    spans = []
    pos = 0
    for i, ln in enumerate(sizes):
        r = i % 2
        if r == 0 and ln > 2048:
            ln = 2048
        spans.append([r, 0, pos, ln])
        pos += ln
    assert pos == TOT, pos
    if split_last:
        r, ro, st, ln = spans.pop()
        spans.append([r, 0, st, 512])
        spans.append([r, 512, st + 512, 512])
    return spans


def build_program() -> bass.Bass:
    nc = bacc.Bacc(
        "TRN2", target_bir_lowering=False, debug=False, num_devices=NCORES
    )

    qt_d = nc.dram_tensor("qt", [HPC, D, S], BF16, kind="ExternalInput")
    kt_d = nc.dram_tensor("kt", [HPC, D, S], BF16, kind="ExternalInput")
    vt_d = nc.dram_tensor("vt", [HPC, D, S], BF16, kind="ExternalInput")
    wqkt_d = nc.dram_tensor("wqkt", [HPC, D, D], BF16, kind="ExternalInput")
    wv_d = nc.dram_tensor("wv", [HPC, D, D], BF16, kind="ExternalInput")
    out_d = nc.dram_tensor("out", [HPC, S, D], F32, kind="ExternalOutput")

    with tile.TileContext(nc) as tc, ExitStack() as ctx:
        sb = ctx.enter_context(tc.tile_pool(name="sb", bufs=1))
        ps = ctx.enter_context(tc.tile_pool(name="ps", bufs=1, space="PSUM"))

        # warm the exp activation-table load under the initial DMAs
        warm = sb.tile([D, 1], F32, tag="warm")
        nc.gpsimd.memset(warm[:], 0.0)
        warm2 = sb.tile([D, 1], BF16, tag="warm2")
        nc.scalar.activation(warm2[:], warm[:], AF.Exp)

        # PSUM: P=4 banks, Q=2 banks, VP scratch=1 bank, ctx=1 bank
        P = ps.tile([D, 2048], F32, tag="P")
        Qr = ps.tile([D, 1024], F32, tag="Q")
        VP = ps.tile([D, 512], F32, tag="VP")
        ctxb = ps.tile([D, 3 * 129], F32, tag="ctx")
        regions = (P, Qr)

        class HeadEmitter:
            def __init__(self, h):
                self.h = h
                self.spans = make_spans(split_last=(h == HPC - 1))
                self.span_i = 0
                self.pv_i = 0
                self.osb = None
                self.wqkt = sb.tile([D, D], BF16, tag="wqkt", bufs=2,
                                    name=f"wqkt_{h}")
                nc.sync.dma_start(self.wqkt[:], wqkt_d[h])
                self.ktb = sb.tile([D, S], BF16, tag="ktb", bufs=2,
                                   name=f"ktb_{h}")
                nc.sync.dma_start(self.ktb[:], kt_d[h])
                self.qtb = sb.tile([D, S], BF16, tag="qtb", bufs=2,
                                   name=f"qtb_{h}")
                nc.sync.dma_start(self.qtb[:], qt_d[h])
                self.wvb = sb.tile([D, D], BF16, tag="wvb", bufs=2,
                                   name=f"wvb_{h}")
                nc.gpsimd.dma_start(self.wvb[:], wv_d[h])
                self.vtb = sb.tile([D, S], BF16, tag="vtb", bufs=2,
                                   name=f"vtb_{h}")
                nc.gpsimd.dma_start(self.vtb[:], vt_d[h])
                self.Mb = sb.tile([D, D], BF16, tag="Mb", bufs=2,
                                  name=f"Mb_{h}")
                self.kmt = sb.tile([D, S], BF16, tag="kmt", bufs=2,
                                   name=f"kmt_{h}")
                self.vsb = sb.tile([D, NB * 129], BF16, tag="vsb", bufs=2,
                                   name=f"vsb_{h}")
                self.vsb3 = self.vsb.rearrange("p (j e) -> p j e", j=NB)
                nc.gpsimd.memset(self.vsb3[:, :, 128:129], 1.0)
                self.expt = sb.tile([D, TOT], BF16, tag="expt", bufs=2,
                                    name=f"expt_{h}")

            def pro_M(self):
                nc.tensor.matmul(VP[:, 0:128], self.wqkt[:], self.wqkt[:])
                nc.vector.tensor_copy(self.Mb[:], VP[:, 0:128])

            def kchunk(self, c):
                sl = slice(512 * c, 512 * (c + 1))
                nc.tensor.matmul(VP[:], self.Mb[:], self.ktb[:, sl])
                nc.vector.tensor_copy(self.kmt[:, sl], VP[:])

            def vround(self, r):
                for m in range(4):
                    j = 4 * r + m
                    nc.tensor.matmul(
                        VP[:, 128 * m : 128 * (m + 1)],
                        self.vtb[:, 128 * j : 128 * (j + 1)],
                        self.wvb[:],
                    )
                nc.vector.tensor_copy(
                    self.vsb3[:, 4 * r : 4 * r + 4, 0:128],
                    VP.rearrange("p (j e) -> p j e", j=4),
                )

            def _fire_span(self, si):
                r, ro, st, ln = self.spans[si]
                reg = regions[r]
                nc.scalar.activation(
                    self.expt[:, st : st + ln],
                    reg[:, ro : ro + ln],
                    AF.Exp,
                    scale=1.0 / NF,
                )
                for j in range(NB):
                    if st <= OFF[j] < st + ln:
                        nc.gpsimd.affine_select(
                            self.expt[:, OFF[j] : OFF[j] + 128],
                            self.expt[:, OFF[j] : OFF[j] + 128],
                            pattern=[[1, 128]],
                            compare_op=mybir.AluOpType.is_ge,
                            fill=0.0,
                            base=0,
                            channel_multiplier=-1,
                        )

            def _emit_pv(self, i):
                h = self.h
                slot = ctxb[:, (i % 3) * 129 : (i % 3) * 129 + 129]
                for j2 in range(i + 1):
                    woff = OFF[j2] + (i - j2) * 128
                    nc.tensor.matmul(
                        slot,
                        self.expt[:, woff : woff + 128],
                        self.vsb[:, 129 * j2 : 129 * j2 + 129],
                        start=(j2 == 0),
                        stop=(j2 == i),
                    )
                if i % 4 == 0:
                    self.osb = sb.tile([D, 512], F32, tag="osb", bufs=2,
                                       name=f"osb_{h}_{i // 4}")
                rec = sb.tile([D, 1], F32, tag="rec", bufs=3,
                              name=f"rec_{h}_{i}")
                nc.vector.reciprocal(rec[:], slot[:, 128:129])
                nc.vector.tensor_scalar_mul(
                    self.osb[:, (i % 4) * 128 : (i % 4) * 128 + 128],
                    slot[:, 0:128],
                    rec[:],
                )
                if i % 4 == 3:
                    g = i // 4
                    nc.sync.dma_start(
                        out_d[h, 512 * g : 512 * (g + 1), :].rearrange(
                            "(b s) e -> s b e", b=4
                        ),
                        self.osb.rearrange("p (b e) -> p b e", b=4),
                    )

            def pv_flush(self):
                done = self.spans[self.span_i - 1][2] + self.spans[self.span_i - 1][3] \
                    if self.span_i > 0 else 0
                while self.pv_i < NB and OFF[self.pv_i] + 128 <= done:
                    self._emit_pv(self.pv_i)
                    self.pv_i += 1

            def span_run(self, a, b, flush=True):
                """Emit score matmuls + exp for spans [a, b)."""
                for si in range(a, b):
                    r, ro, st, ln = self.spans[si]
                    reg = regions[r]
                    pos = st
                    while pos < st + ln:
                        # current k-block
                        j = 0
                        while OFF[j + 1] <= pos:
                            j += 1
                        col = ro + (pos - st)
                        nxt = min(
                            OFF[j + 1],
                            st + ln,
                            pos + (512 - (col % 512)),
                        )
                        nc.tensor.matmul(
                            reg[:, col : col + (nxt - pos)],
                            self.kmt[:, 128 * j : 128 * (j + 1)],
                            self.qtb[
                                :,
                                128 * j + (pos - OFF[j]) : 128 * j + (nxt - OFF[j]),
                            ],
                        )
                        pos = nxt
                    self._fire_span(si)
                    self.span_i = si + 1
                    if flush:
                        self.pv_flush()

        e0 = HeadEmitter(0)
        e1 = HeadEmitter(1)

        # ---- software-pipelined drive of the two heads -------------------
        e0.pro_M()
        e0.kchunk(0)
        e0.span_run(0, 1, flush=False)
        e0.vround(0)
        e0.kchunk(1)
        e0.span_run(1, 2)
        e0.vround(1)
        e0.span_run(2, 3)
        e0.vround(2)
        e0.span_run(3, 4)
        e0.vround(3)
        e0.span_run(4, 5)
        e0.kchunk(2)
        e0.span_run(5, 7)
        e0.kchunk(3)
        e0.span_run(7, 9)
        e1.pro_M()
        e1.kchunk(0)
        e0.span_run(9, 10)
        e1.kchunk(1)
        e0.span_run(10, 11)
        e1.kchunk(2)
        e0.span_run(11, 12, flush=False)
        e1.kchunk(3)
        e1.span_run(0, 1, flush=False)
        e0.pv_flush()  # head0 tail regions
        e1.vround(0)
        e1.span_run(1, 2)
        e1.vround(1)
        e1.span_run(2, 3)
        e1.vround(2)
        e1.span_run(3, 4)
        e1.vround(3)
        e1.span_run(4, len(e1.spans))
        e1.pv_flush()

    nc.compile()
    return nc


_NC_CACHE = None


def _get_program():
    global _NC_CACHE
    if _NC_CACHE is None:
        _NC_CACHE = build_program()
    return _NC_CACHE


def make_in_maps(query_layer, key_layer, value_layer, svd_qk, svd_v):
    bf = ml_dtypes.bfloat16
    qt = np.ascontiguousarray(
        np.asarray(query_layer)[:, 0].transpose(1, 2, 0).astype(bf)
    )
    kt = np.ascontiguousarray(
        np.asarray(key_layer)[:, 0].transpose(1, 2, 0).astype(bf)
    )
    vt = np.ascontiguousarray(
        np.asarray(value_layer)[:, 0].transpose(1, 2, 0).astype(bf)
    )
    wqkt = np.ascontiguousarray(
        np.asarray(svd_qk).transpose(0, 2, 1).astype(bf)
    )
    wv = np.ascontiguousarray(np.asarray(svd_v).astype(bf))

    in_maps = []
    for c in range(NCORES):
        hs = slice(c * HPC, (c + 1) * HPC)
        in_maps.append(
            {
                "qt": qt[hs],
                "kt": kt[hs],
                "vt": vt[hs],
                "wqkt": wqkt[hs],
                "wv": wv[hs],
            }
        )
    return in_maps


def assemble_output(results):
    out = np.empty((S, B, H * D), dtype=np.float32)
    for c in range(NCORES):
        o = results[c]["out"]  # [HPC, S, D]
        for hl in range(HPC):
            h = c * HPC + hl
            out[:, 0, h * D : (h + 1) * D] = o[hl]
    return out


def kernel(query_layer, key_layer, value_layer, attention_mask, svd_qk, svd_v):
    nc = _get_program()
    in_maps = make_in_maps(query_layer, key_layer, value_layer, svd_qk, svd_v)
    res = run_bass_kernel_spmd(nc, in_maps, list(range(NCORES))).results
    return assemble_output(res)


# revision 6
# speedup vs baseline: 1.7996x; 1.0813x over previous
"""Trainium2 Bass kernel for nn_CoreAttention (S=2048, B=1, H=16, D=128).

Sharding: 16 heads across 8 NeuronCores (2 heads/core, tensor parallel).

Per head, fully fused causal attention:
    M      = Wqk Wqk^T                  (PE, one matmul; M is symmetric)
    kmt    = M K^T                      (PE, 4 matmuls ping-ponged through a
                                         1-bank PSUM scratch; q side stays RAW)
    v      = V_block @ Wv               (PE, 16 matmuls -> [s,e] chunks)
    scoresT[k,q] = kmt_j^T @ Q^T        (PE, causal only, streams into two
                                         big PSUM spans: P=4 banks, Q=2)
    expT   = exp(scoresT / NF)          (ACT, 12 big instrs/head, -> SBUF)
    mask   = affine_select on diagonal  (GPSIMD, zero strict lower triangle)
    ctx[q,(e|sum)] = sum_j expT_j^T @ [v_j | 1]   (PE, expT-stationary,
                                         129-wide rhs; col 128 = softmax sum)
    out    = ctx * (1/sum)              (DVE reciprocal + per-partition mul)

The two heads are software-pipelined: head1's projections are interleaved
into head0's span loop so ACT/PE never drain at the head seam. Input DMAs
are issued up-front on two queues (sync + gpsimd). No transposes, no
separate softmax-sum pass, no device-side casts (host supplies bf16
pre-transposed tensors). exp runs without max-subtraction: scores/NF ~
N(0,1), so exp stays in [e-6, e+6].
"""

import sys
from contextlib import ExitStack

import numpy as np

for _p in ("/opt/trn_rl_repo",):
    if _p not in sys.path:
        sys.path.insert(0, _p)

import ml_dtypes
import concourse.bass as bass
import concourse.tile as tile
from concourse import bacc, mybir
from concourse.bass_utils import run_bass_kernel_spmd

S, B, H, D = 2048, 1, 16, 128
HPC = 2  # heads per core
NCORES = 8
NB = S // 128  # 16 k-blocks of 128
NF = float(np.sqrt(2048.0 / 16.0))  # NORM_FACTOR
TOT = 17408  # total causal score columns

F32 = mybir.dt.float32
BF16 = mybir.dt.bfloat16
AF = mybir.ActivationFunctionType

# block start offsets in the concatenated causal score stream
OFF = [0]
for j in range(NB):
    OFF.append(OFF[-1] + (S - 128 * j))
assert OFF[-1] == TOT


def make_spans(split_last: bool):
    """(region_idx, region_off, start, len) alternating P(2048) / Q(1024)."""
    sizes = [2048, 1024] * 4 + [2048, 1024, 1024, 1024]
    spans = []
    pos = 0
    for i, ln in enumerate(sizes):
        r = i % 2
        if r == 0 and ln > 2048:
            ln = 2048
        spans.append([r, 0, pos, ln])
        pos += ln
    assert pos == TOT, pos
    if split_last:
        r, ro, st, ln = spans.pop()
        spans.append([r, 0, st, 512])
        spans.append([r, 512, st + 512, 512])
    return spans


def build_program() -> bass.Bass:
    nc = bacc.Bacc(
        "TRN2", target_bir_lowering=False, debug=False, num_devices=NCORES
    )

    qt_d = nc.dram_tensor("qt", [HPC, D, S], BF16, kind="ExternalInput")
    kt_d = nc.dram_tensor("kt", [HPC, D, S], BF16, kind="ExternalInput")
    vt_d = nc.dram_tensor("vt", [HPC, D, S], BF16, kind="ExternalInput")
    wqkt_d = nc.dram_tensor("wqkt", [HPC, D, D], BF16, kind="ExternalInput")
    wv_d = nc.dram_tensor("wv", [HPC, D, D], BF16, kind="ExternalInput")
    out_d = nc.dram_tensor("out", [HPC, S, D], F32, kind="ExternalOutput")

    with tile.TileContext(nc) as tc, ExitStack() as ctx:
        sb = ctx.enter_context(tc.tile_pool(name="sb", bufs=1))
        ps = ctx.enter_context(tc.tile_pool(name="ps", bufs=1, space="PSUM"))

        # warm the exp activation-table load under the initial DMAs
        warm = sb.tile([D, 1], F32, tag="warm")
        nc.gpsimd.memset(warm[:], 0.0)
        warm2 = sb.tile([D, 1], BF16, tag="warm2")
        nc.scalar.activation(warm2[:], warm[:], AF.Exp)

        # PSUM: P=4 banks, Q=2 banks, VP scratch=1 bank, ctx=1 bank
        P = ps.tile([D, 2048], F32, tag="P")
        Qr = ps.tile([D, 1024], F32, tag="Q")
        VP = ps.tile([D, 512], F32, tag="VP")
        ctxb = ps.tile([D, 3 * 129], F32, tag="ctx")
        regions = (P, Qr)

        class HeadEmitter:
            def __init__(self, h):
                self.h = h
                self.spans = make_spans(split_last=(h == HPC - 1))
                self.span_i = 0
                self.pv_i = 0
                self.osb = None
                self.wqkt = sb.tile([D, D], BF16, tag="wqkt", bufs=2,
                                    name=f"wqkt_{h}")
                nc.sync.dma_start(self.wqkt[:], wqkt_d[h])
                self.ktb = sb.tile([D, S], BF16, tag="ktb", bufs=2,
                                   name=f"ktb_{h}")
                nc.sync.dma_start(self.ktb[:], kt_d[h])
                self.qtb = sb.tile([D, S], BF16, tag="qtb", bufs=2,
                                   name=f"qtb_{h}")
                # head0's q load rides the otherwise-idle scalar queue so
                # span0 scores can start as early as possible
                if h == 0:
                    nc.scalar.dma_start(self.qtb[:], qt_d[h])
                else:
                    nc.sync.dma_start(self.qtb[:], qt_d[h])
                self.wvb = sb.tile([D, D], BF16, tag="wvb", bufs=2,
                                   name=f"wvb_{h}")
                nc.gpsimd.dma_start(self.wvb[:], wv_d[h])
                self.vtb = sb.tile([D, S], BF16, tag="vtb", bufs=2,
                                   name=f"vtb_{h}")
                nc.gpsimd.dma_start(self.vtb[:], vt_d[h])
                self.Mb = sb.tile([D, D], BF16, tag="Mb", bufs=2,
                                  name=f"Mb_{h}")
                self.kmt = sb.tile([D, S], BF16, tag="kmt", bufs=2,
                                   name=f"kmt_{h}")
                self.vsb = sb.tile([D, NB * 129], BF16, tag="vsb", bufs=2,
                                   name=f"vsb_{h}")
                self.vsb3 = self.vsb.rearrange("p (j e) -> p j e", j=NB)
                nc.gpsimd.memset(self.vsb3[:, :, 128:129], 1.0)
                self.expt = sb.tile([D, TOT], BF16, tag="expt", bufs=2,
                                    name=f"expt_{h}")

            def pro_M(self):
                nc.tensor.matmul(VP[:, 0:128], self.wqkt[:], self.wqkt[:])
                nc.vector.tensor_copy(self.Mb[:], VP[:, 0:128])

            def kchunk(self, c):
                sl = slice(512 * c, 512 * (c + 1))
                nc.tensor.matmul(VP[:], self.Mb[:], self.ktb[:, sl])
                nc.vector.tensor_copy(self.kmt[:, sl], VP[:])

            def vround(self, r):
                for m in range(4):
                    j = 4 * r + m
                    nc.tensor.matmul(
                        VP[:, 128 * m : 128 * (m + 1)],
                        self.vtb[:, 128 * j : 128 * (j + 1)],
                        self.wvb[:],
                    )
                nc.vector.tensor_copy(
                    self.vsb3[:, 4 * r : 4 * r + 4, 0:128],
                    VP.rearrange("p (j e) -> p j e", j=4),
                )

            def _fire_span(self, si):
                r, ro, st, ln = self.spans[si]
                reg = regions[r]
                nc.scalar.activation(
                    self.expt[:, st : st + ln],
                    reg[:, ro : ro + ln],
                    AF.Exp,
                    scale=1.0 / NF,
                )
                for j in range(NB):
                    if st <= OFF[j] < st + ln:
                        nc.gpsimd.affine_select(
                            self.expt[:, OFF[j] : OFF[j] + 128],
                            self.expt[:, OFF[j] : OFF[j] + 128],
                            pattern=[[1, 128]],
                            compare_op=mybir.AluOpType.is_ge,
                            fill=0.0,
                            base=0,
                            channel_multiplier=-1,
                        )

            def _emit_pv(self, i):
                h = self.h
                slot = ctxb[:, (i % 3) * 129 : (i % 3) * 129 + 129]
                for j2 in range(i + 1):
                    woff = OFF[j2] + (i - j2) * 128
                    nc.tensor.matmul(
                        slot,
                        self.expt[:, woff : woff + 128],
                        self.vsb[:, 129 * j2 : 129 * j2 + 129],
                        start=(j2 == 0),
                        stop=(j2 == i),
                    )
                if i % 4 == 0:
                    self.osb = sb.tile([D, 512], F32, tag="osb", bufs=2,
                                       name=f"osb_{h}_{i // 4}")
                    self.ctxs = sb.tile([D, 4 * 129], F32, tag="ctxs", bufs=2,
                                        name=f"ctxs_{h}_{i // 4}")
                # single fast copy releases the PSUM slot; normalization is
                # batched per group of 4 off the critical path
                ctxs3 = self.ctxs.rearrange("p (r e) -> p r e", r=4)
                nc.vector.tensor_copy(ctxs3[:, i % 4, :], slot)
                if i % 4 == 3:
                    g = i // 4
                    rec = sb.tile([D, 4], F32, tag="rec", bufs=2,
                                  name=f"rec_{h}_{g}")
                    nc.vector.reciprocal(rec[:], ctxs3[:, :, 128])
                    for r in range(4):
                        nc.vector.tensor_scalar_mul(
                            self.osb[:, r * 128 : r * 128 + 128],
                            ctxs3[:, r, 0:128],
                            rec[:, r : r + 1],
                        )
                    nc.sync.dma_start(
                        out_d[h, 512 * g : 512 * (g + 1), :].rearrange(
                            "(b s) e -> s b e", b=4
                        ),
                        self.osb.rearrange("p (b e) -> p b e", b=4),
                    )

            def pv_flush(self):
                done = self.spans[self.span_i - 1][2] + self.spans[self.span_i - 1][3] \
                    if self.span_i > 0 else 0
                while self.pv_i < NB and OFF[self.pv_i] + 128 <= done:
                    self._emit_pv(self.pv_i)
                    self.pv_i += 1

            def span_run(self, a, b, flush=True):
                """Emit score matmuls + exp for spans [a, b)."""
                for si in range(a, b):
                    r, ro, st, ln = self.spans[si]
                    reg = regions[r]
                    pos = st
                    while pos < st + ln:
                        # current k-block
                        j = 0
                        while OFF[j + 1] <= pos:
                            j += 1
                        col = ro + (pos - st)
                        nxt = min(
                            OFF[j + 1],
                            st + ln,
                            pos + (512 - (col % 512)),
                        )
                        nc.tensor.matmul(
                            reg[:, col : col + (nxt - pos)],
                            self.kmt[:, 128 * j : 128 * (j + 1)],
                            self.qtb[
                                :,
                                128 * j + (pos - OFF[j]) : 128 * j + (nxt - OFF[j]),
                            ],
                        )
                        pos = nxt
                    self._fire_span(si)
                    self.span_i = si + 1
                    if flush:
                        self.pv_flush()

        e0 = HeadEmitter(0)
        e1 = HeadEmitter(1)

        # ---- software-pipelined drive of the two heads -------------------
        e0.pro_M()
        e0.kchunk(0)
        e0.span_run(0, 1, flush=False)
        e0.vround(0)
        e0.kchunk(1)
        e0.span_run(1, 2)
        e0.vround(1)
        e0.span_run(2, 3)
        e0.vround(2)
        e0.span_run(3, 4)
        e0.vround(3)
        e0.span_run(4, 5)
        e0.kchunk(2)
        e0.span_run(5, 7)
        e0.kchunk(3)
        e0.span_run(7, 9)
        e1.pro_M()
        e1.kchunk(0)
        e0.span_run(9, 10)
        e1.kchunk(1)
        e0.span_run(10, 11)
        e1.kchunk(2)
        e0.span_run(11, 12, flush=False)
        e1.kchunk(3)
        e1.span_run(0, 1, flush=False)
        e0.pv_flush()  # head0 tail regions
        e1.vround(0)
        e1.span_run(1, 2)
        e1.vround(1)
        e1.span_run(2, 3)
        e1.vround(2)
        e1.span_run(3, 4)
        e1.vround(3)
        e1.span_run(4, len(e1.spans))
        e1.pv_flush()

    nc.compile()
    return nc


_NC_CACHE = None


def _get_program():
    global _NC_CACHE
    if _NC_CACHE is None:
        _NC_CACHE = build_program()
    return _NC_CACHE


def make_in_maps(query_layer, key_layer, value_layer, svd_qk, svd_v):
    bf = ml_dtypes.bfloat16
    qt = np.ascontiguousarray(
        np.asarray(query_layer)[:, 0].transpose(1, 2, 0).astype(bf)
    )
    kt = np.ascontiguousarray(
        np.asarray(key_layer)[:, 0].transpose(1, 2, 0).astype(bf)
    )
    vt = np.ascontiguousarray(
        np.asarray(value_layer)[:, 0].transpose(1, 2, 0).astype(bf)
    )
    wqkt = np.ascontiguousarray(
        np.asarray(svd_qk).transpose(0, 2, 1).astype(bf)
    )
    wv = np.ascontiguousarray(np.asarray(svd_v).astype(bf))

    in_maps = []
    for c in range(NCORES):
        hs = slice(c * HPC, (c + 1) * HPC)
        in_maps.append(
            {
                "qt": qt[hs],
                "kt": kt[hs],
                "vt": vt[hs],
                "wqkt": wqkt[hs],
                "wv": wv[hs],
            }
        )
    return in_maps


def assemble_output(results):
    out = np.empty((S, B, H * D), dtype=np.float32)
    for c in range(NCORES):
        o = results[c]["out"]  # [HPC, S, D]
        for hl in range(HPC):
            h = c * HPC + hl
            out[:, 0, h * D : (h + 1) * D] = o[hl]
    return out


def kernel(query_layer, key_layer, value_layer, attention_mask, svd_qk, svd_v):
    nc = _get_program()
    in_maps = make_in_maps(query_layer, key_layer, value_layer, svd_qk, svd_v)
    res = run_bass_kernel_spmd(nc, in_maps, list(range(NCORES))).results
    return assemble_output(res)


# revision 9
# speedup vs baseline: 1.8345x; 1.0194x over previous
"""Trainium2 Bass kernel for nn_CoreAttention (S=2048, B=1, H=16, D=128).

Sharding: 16 heads across 8 NeuronCores (2 heads/core, tensor parallel).

Per head, fully fused causal attention:
    M      = Wqk Wqk^T                  (PE, one matmul; M is symmetric)
    kmt    = M K^T                      (PE, 4 matmuls ping-ponged through a
                                         1-bank PSUM scratch; q side stays RAW)
    v      = V_block @ Wv               (PE, 16 matmuls -> [s,e] chunks)
    scoresT[k,q] = kmt_j^T @ Q^T        (PE, causal only, streams into two
                                         big PSUM spans: P=4 banks, Q=2)
    expT   = exp(scoresT / NF)          (ACT, 12 big instrs/head, -> SBUF)
    mask   = affine_select on diagonal  (GPSIMD, zero strict lower triangle)
    ctx[q,(e|sum)] = sum_j expT_j^T @ [v_j | 1]   (PE, expT-stationary,
                                         129-wide rhs; col 128 = softmax sum)
    out    = ctx * (1/sum)              (DVE reciprocal + per-partition mul)

The two heads are software-pipelined: head1's projections are interleaved
into head0's span loop so ACT/PE never drain at the head seam. Input DMAs
are issued up-front on two queues (sync + gpsimd). No transposes, no
separate softmax-sum pass, no device-side casts (host supplies bf16
pre-transposed tensors). exp runs without max-subtraction: scores/NF ~
N(0,1), so exp stays in [e-6, e+6].
"""

import sys
from contextlib import ExitStack

import numpy as np

for _p in ("/opt/trn_rl_repo",):
    if _p not in sys.path:
        sys.path.insert(0, _p)

import ml_dtypes
import concourse.bass as bass
import concourse.tile as tile
from concourse import bacc, mybir
from concourse.bass_utils import run_bass_kernel_spmd

S, B, H, D = 2048, 1, 16, 128
HPC = 2  # heads per core
NCORES = 8
NB = S // 128  # 16 k-blocks of 128
NF = float(np.sqrt(2048.0 / 16.0))  # NORM_FACTOR
TOT = 17408  # total causal score columns

F32 = mybir.dt.float32
BF16 = mybir.dt.bfloat16
AF = mybir.ActivationFunctionType

# block start offsets in the concatenated causal score stream
OFF = [0]
for j in range(NB):
    OFF.append(OFF[-1] + (S - 128 * j))
assert OFF[-1] == TOT


def make_spans(split_last: bool):
    """(region_idx, region_off, start, len) alternating P(2048) / Q(1024)."""
    sizes = [2048, 1024] * 4 + [2048, 1024, 1024, 1024]
    spans = []
    pos = 0
    for i, ln in enumerate(sizes):
        r = i % 2
        if r == 0 and ln > 2048:
            ln = 2048
        spans.append([r, 0, pos, ln])
        pos += ln
    assert pos == TOT, pos
    if split_last:
        r, ro, st, ln = spans.pop()
        spans.append([r, 0, st, 512])
        spans.append([r, 512, st + 512, 512])
    return spans


def build_program() -> bass.Bass:
    nc = bacc.Bacc(
        "TRN2", target_bir_lowering=False, debug=False, num_devices=NCORES
    )

    qt_d = nc.dram_tensor("qt", [HPC, D, S], BF16, kind="ExternalInput")
    kt_d = nc.dram_tensor("kt", [HPC, D, S], BF16, kind="ExternalInput")
    vt_d = nc.dram_tensor("vt", [HPC, D, S], BF16, kind="ExternalInput")
    wqkt_d = nc.dram_tensor("wqkt", [HPC, D, D], BF16, kind="ExternalInput")
    wv_d = nc.dram_tensor("wv", [HPC, D, D], BF16, kind="ExternalInput")
    out_d = nc.dram_tensor("out", [HPC, S, D], F32, kind="ExternalOutput")

    with tile.TileContext(nc) as tc, ExitStack() as ctx:
        sb = ctx.enter_context(tc.tile_pool(name="sb", bufs=1))
        ps = ctx.enter_context(tc.tile_pool(name="ps", bufs=1, space="PSUM"))

        # warm the exp activation-table load under the initial DMAs
        warm = sb.tile([D, 1], F32, tag="warm")
        nc.gpsimd.memset(warm[:], 0.0)
        warm2 = sb.tile([D, 1], BF16, tag="warm2")
        nc.scalar.activation(warm2[:], warm[:], AF.Exp)

        # PSUM: P=4 banks, Q=2 banks, VP scratch=1 bank, ctx=1 bank
        P = ps.tile([D, 2048], F32, tag="P")
        Qr = ps.tile([D, 1024], F32, tag="Q")
        VP = ps.tile([D, 512], F32, tag="VP")
        ctxb = ps.tile([D, 3 * 129], F32, tag="ctx")
        regions = (P, Qr)

        class HeadEmitter:
            def __init__(self, h):
                self.h = h
                self.spans = make_spans(split_last=(h == HPC - 1))
                self.span_i = 0
                self.pv_i = 0
                self.osb = None
                self.wqkt = sb.tile([D, D], BF16, tag="wqkt", bufs=2,
                                    name=f"wqkt_{h}")
                nc.sync.dma_start(self.wqkt[:], wqkt_d[h])
                self.ktb = sb.tile([D, S], BF16, tag="ktb", bufs=2,
                                   name=f"ktb_{h}")
                for c in range(2):
                    sl = slice(1024 * c, 1024 * (c + 1))
                    nc.sync.dma_start(self.ktb[:, sl], kt_d[h][:, sl])
                self.qtb = sb.tile([D, S], BF16, tag="qtb", bufs=2,
                                   name=f"qtb_{h}")
                # head0's q load rides the otherwise-idle scalar queue so
                # span0 scores can start as early as possible
                qeng = nc.scalar if h == 0 else nc.sync
                for c in range(2):
                    sl = slice(1024 * c, 1024 * (c + 1))
                    qeng.dma_start(self.qtb[:, sl], qt_d[h][:, sl])
                self.wvb = sb.tile([D, D], BF16, tag="wvb", bufs=2,
                                   name=f"wvb_{h}")
                nc.gpsimd.dma_start(self.wvb[:], wv_d[h])
                self.vtb = sb.tile([D, S], BF16, tag="vtb", bufs=2,
                                   name=f"vtb_{h}")
                for c in range(2):
                    sl = slice(1024 * c, 1024 * (c + 1))
                    nc.gpsimd.dma_start(self.vtb[:, sl], vt_d[h][:, sl])
                self.Mb = sb.tile([D, D], BF16, tag="Mb", bufs=2,
                                  name=f"Mb_{h}")
                self.kmt = sb.tile([D, S], BF16, tag="kmt", bufs=2,
                                   name=f"kmt_{h}")
                self.vsb = sb.tile([D, NB * 129], BF16, tag="vsb", bufs=2,
                                   name=f"vsb_{h}")
                self.vsb3 = self.vsb.rearrange("p (j e) -> p j e", j=NB)
                nc.gpsimd.memset(self.vsb3[:, :, 128:129], 1.0)
                self.expt = sb.tile([D, TOT], BF16, tag="expt", bufs=2,
                                    name=f"expt_{h}")

            def pro_M(self):
                nc.tensor.matmul(VP[:, 0:128], self.wqkt[:], self.wqkt[:])
                nc.vector.tensor_copy(self.Mb[:], VP[:, 0:128])

            def kchunk(self, c):
                sl = slice(512 * c, 512 * (c + 1))
                nc.tensor.matmul(VP[:], self.Mb[:], self.ktb[:, sl])
                nc.vector.tensor_copy(self.kmt[:, sl], VP[:])

            def vround(self, r):
                for m in range(4):
                    j = 4 * r + m
                    nc.tensor.matmul(
                        VP[:, 128 * m : 128 * (m + 1)],
                        self.vtb[:, 128 * j : 128 * (j + 1)],
                        self.wvb[:],
                    )
                nc.vector.tensor_copy(
                    self.vsb3[:, 4 * r : 4 * r + 4, 0:128],
                    VP.rearrange("p (j e) -> p j e", j=4),
                )

            def _fire_span(self, si):
                r, ro, st, ln = self.spans[si]
                reg = regions[r]
                nc.scalar.activation(
                    self.expt[:, st : st + ln],
                    reg[:, ro : ro + ln],
                    AF.Exp,
                    scale=1.0 / NF,
                )
                for j in range(NB):
                    if st <= OFF[j] < st + ln:
                        nc.gpsimd.affine_select(
                            self.expt[:, OFF[j] : OFF[j] + 128],
                            self.expt[:, OFF[j] : OFF[j] + 128],
                            pattern=[[1, 128]],
                            compare_op=mybir.AluOpType.is_ge,
                            fill=0.0,
                            base=0,
                            channel_multiplier=-1,
                        )

            def _emit_pv(self, i):
                h = self.h
                slot = ctxb[:, (i % 3) * 129 : (i % 3) * 129 + 129]
                for j2 in range(i + 1):
                    woff = OFF[j2] + (i - j2) * 128
                    nc.tensor.matmul(
                        slot,
                        self.expt[:, woff : woff + 128],
                        self.vsb[:, 129 * j2 : 129 * j2 + 129],
                        start=(j2 == 0),
                        stop=(j2 == i),
                    )
                if i % 4 == 0:
                    self.osb = sb.tile([D, 512], F32, tag="osb", bufs=2,
                                       name=f"osb_{h}_{i // 4}")
                    self.ctxs = sb.tile([D, 4 * 129], F32, tag="ctxs", bufs=2,
                                        name=f"ctxs_{h}_{i // 4}")
                # single fast copy releases the PSUM slot; normalization is
                # batched per group of 4 off the critical path
                ctxs3 = self.ctxs.rearrange("p (r e) -> p r e", r=4)
                nc.vector.tensor_copy(ctxs3[:, i % 4, :], slot)
                last_grp = (h == HPC - 1) and i >= 12
                if last_grp:
                    # final group of the final head: normalize + store per
                    # region so the kernel tail is as short as possible
                    r = i % 4
                    rec = sb.tile([D, 1], F32, tag="rec1", bufs=2,
                                  name=f"rec1_{h}_{i}")
                    nc.vector.reciprocal(rec[:], ctxs3[:, r, 128:129])
                    nc.vector.tensor_scalar_mul(
                        self.osb[:, r * 128 : r * 128 + 128],
                        ctxs3[:, r, 0:128],
                        rec[:],
                    )
                    nc.sync.dma_start(
                        out_d[h, 128 * i : 128 * (i + 1), :],
                        self.osb[:, r * 128 : r * 128 + 128],
                    )
                elif i % 4 == 3:
                    g = i // 4
                    rec = sb.tile([D, 4], F32, tag="rec", bufs=2,
                                  name=f"rec_{h}_{g}")
                    nc.vector.reciprocal(rec[:], ctxs3[:, :, 128])
                    for r in range(4):
                        nc.vector.tensor_scalar_mul(
                            self.osb[:, r * 128 : r * 128 + 128],
                            ctxs3[:, r, 0:128],
                            rec[:, r : r + 1],
                        )
                    nc.sync.dma_start(
                        out_d[h, 512 * g : 512 * (g + 1), :].rearrange(
                            "(b s) e -> s b e", b=4
                        ),
                        self.osb.rearrange("p (b e) -> p b e", b=4),
                    )

            def pv_flush(self):
                done = self.spans[self.span_i - 1][2] + self.spans[self.span_i - 1][3] \
                    if self.span_i > 0 else 0
                while self.pv_i < NB and OFF[self.pv_i] + 128 <= done:
                    self._emit_pv(self.pv_i)
                    self.pv_i += 1

            def span_run(self, a, b, flush=True):
                """Emit score matmuls + exp for spans [a, b)."""
                for si in range(a, b):
                    r, ro, st, ln = self.spans[si]
                    reg = regions[r]
                    pos = st
                    while pos < st + ln:
                        # current k-block
                        j = 0
                        while OFF[j + 1] <= pos:
                            j += 1
                        col = ro + (pos - st)
                        nxt = min(
                            OFF[j + 1],
                            st + ln,
                            pos + (512 - (col % 512)),
                        )
                        nc.tensor.matmul(
                            reg[:, col : col + (nxt - pos)],
                            self.kmt[:, 128 * j : 128 * (j + 1)],
                            self.qtb[
                                :,
                                128 * j + (pos - OFF[j]) : 128 * j + (nxt - OFF[j]),
                            ],
                        )
                        pos = nxt
                    self._fire_span(si)
                    self.span_i = si + 1
                    if flush:
                        self.pv_flush()

        e0 = HeadEmitter(0)
        e1 = HeadEmitter(1)

        # ---- software-pipelined drive of the two heads -------------------
        e0.pro_M()
        e0.kchunk(0)
        e0.span_run(0, 1, flush=False)
        e0.vround(0)
        e0.kchunk(1)
        e0.span_run(1, 2)
        e0.vround(1)
        e0.span_run(2, 3)
        e0.vround(2)
        e0.span_run(3, 4)
        e0.vround(3)
        e0.span_run(4, 5)
        e0.kchunk(2)
        e0.span_run(5, 6)
        e0.kchunk(3)
        e0.span_run(6, 7)
        e1.pro_M()
        e0.span_run(7, 8)
        e1.kchunk(0)
        e0.span_run(8, 9)
        e1.kchunk(1)
        e0.span_run(9, 10)
        e1.kchunk(2)
        e0.span_run(10, 11)
        e1.kchunk(3)
        e0.span_run(11, 12, flush=False)
        e1.span_run(0, 1, flush=False)
        e0.pv_flush()  # head0 tail regions
        e1.vround(0)
        e1.span_run(1, 2)
        e1.vround(1)
        e1.span_run(2, 3)
        e1.vround(2)
        e1.span_run(3, 4)
        e1.vround(3)
        e1.span_run(4, len(e1.spans))
        e1.pv_flush()

    nc.compile()
    return nc


_NC_CACHE = None


def _get_program():
    global _NC_CACHE
    if _NC_CACHE is None:
        _NC_CACHE = build_program()
    return _NC_CACHE


def make_in_maps(query_layer, key_layer, value_layer, svd_qk, svd_v):
    bf = ml_dtypes.bfloat16
    qt = np.ascontiguousarray(
        np.asarray(query_layer)[:, 0].transpose(1, 2, 0).astype(bf)
    )
    kt = np.ascontiguousarray(
        np.asarray(key_layer)[:, 0].transpose(1, 2, 0).astype(bf)
    )
    vt = np.ascontiguousarray(
        np.asarray(value_layer)[:, 0].transpose(1, 2, 0).astype(bf)
    )
    wqkt = np.ascontiguousarray(
        np.asarray(svd_qk).transpose(0, 2, 1).astype(bf)
    )
    wv = np.ascontiguousarray(np.asarray(svd_v).astype(bf))

    in_maps = []
    for c in range(NCORES):
        hs = slice(c * HPC, (c + 1) * HPC)
        in_maps.append(
            {
                "qt": qt[hs],
                "kt": kt[hs],
                "vt": vt[hs],
                "wqkt": wqkt[hs],
                "wv": wv[hs],
            }
        )
    return in_maps


def assemble_output(results):
    out = np.empty((S, B, H * D), dtype=np.float32)
    for c in range(NCORES):
        o = results[c]["out"]  # [HPC, S, D]
        for hl in range(HPC):
            h = c * HPC + hl
            out[:, 0, h * D : (h + 1) * D] = o[hl]
    return out


def kernel(query_layer, key_layer, value_layer, attention_mask, svd_qk, svd_v):
    nc = _get_program()
    in_maps = make_in_maps(query_layer, key_layer, value_layer, svd_qk, svd_v)
    res = run_bass_kernel_spmd(nc, in_maps, list(range(NCORES))).results
    return assemble_output(res)
